# revision 51
# baseline (speedup 1.0000x reference)
"""Causal multi-head self-attention (S=4096, D=1024, H=16, RoPE) on 8 trn2 cores.

Tensor-parallel over heads: core c owns heads 2c, 2c+1.
Pipeline per core:
  A) dma-transpose x slices on demand (full x shipped to every core)
  B) qT/kT projections in transposed+feature-grouped layout, RoPE, v natural
  C) flash-style causal attention with transposed scores; exp on ACT;
     denominator via ones-column in v; normalize -> headsT [128, 4096]
  D) range-wise AllGather of headsT -> output projection for a 128-col
     slice of out, overlapped with attention of later tiles
Host assembles out[0, :, 128c:128c+128] = outT_c.T
"""

import sys

for _p in ("/opt/trn_rl_repo", "/root/.axon_site/_ro/trn_rl_repo"):
    if _p not in sys.path:
        sys.path.append(_p)

import numpy as np
import ml_dtypes

import concourse.bass as bass
import concourse.tile as tile
from concourse import bacc, mybir
from concourse.bass_utils import run_bass_kernel_spmd

BF16 = mybir.dt.bfloat16
F32 = mybir.dt.float32
F32R = mybir.dt.float32r
NPBF16 = ml_dtypes.bfloat16

S = 4096          # sequence
D = 1024          # model dim
NH = 16           # heads
DK = 64           # head dim
NCORES = 8
HPC = NH // NCORES          # 2 heads per core
P = HPC * DK                # 128 = per-core head feature count
THETA = 10000.0
ST = 512                    # s-tile width (a-tile width too)
NT = S // ST                # 8 tiles
EXPFN = mybir.ActivationFunctionType.Exp

# heads-allgather ranges: (first s-tile, n s-tiles); finer at the end so the
# final gather+out-proj tail is short
RANGES = [(0, 2), (2, 2), (4, 2), (6, 1), (7, 1)]
GATHER_AT = {1: 0, 3: 1, 5: 2, 6: 3, 7: 4}   # t -> range k gathered after attn(t)
OUTPROJ_AT = {3: 0, 5: 1, 6: 2, 7: 3}        # t -> range k projected after attn(t)

_CACHE = {}


def _build_program():
    import concourse.bass_interp as _bi

    _orig_sim = _bi.CoreSim.simulate

    def _rec(self, *a, **k):
        r = _orig_sim(self, *a, **k)
        try:
            _CACHE["predicted_ns"] = float(self.time)
        except Exception:
            pass
        return r

    _bi.CoreSim.simulate = _rec
    try:
        return _build_program_inner()
    finally:
        _bi.CoreSim.simulate = _orig_sim


def _build_program_inner():
    nc = bacc.Bacc(
        "TRN2", target_bir_lowering=False, debug=False, num_devices=NCORES
    )

    # ---- I/O ----
    xs = nc.dram_tensor("xs", [S, D], BF16, kind="ExternalInput").ap()
    wqT = nc.dram_tensor("wqT", [128, D], BF16, kind="ExternalInput").ap()
    wkT = nc.dram_tensor("wkT", [128, D], BF16, kind="ExternalInput").ap()
    wvT = nc.dram_tensor("wvT", [128, D], BF16, kind="ExternalInput").ap()
    woT = nc.dram_tensor("woT", [128, D], BF16, kind="ExternalInput").ap()
    cosg = nc.dram_tensor("cosg", [P, S], F32, kind="ExternalInput").ap()
    sing = nc.dram_tensor("sing", [P, S], F32, kind="ExternalInput").ap()
    masks = nc.dram_tensor("masks", [128, 1408], BF16, kind="ExternalInput").ap()
    ones2 = nc.dram_tensor("ones2", [33, 128], F32, kind="ExternalInput").ap()
    outT = nc.dram_tensor("outT", [P, S], F32, kind="ExternalOutput").ap()

    cc_ho_in = [
        nc.dram_tensor(f"cc_ho_in{k}", [P, n * ST], BF16)
        for k, (_, n) in enumerate(RANGES)
    ]
    cc_ho_out = [
        nc.dram_tensor(f"cc_ho_out{k}", [P * NCORES, n * ST], BF16, addr_space="Shared")
        for k, (_, n) in enumerate(RANGES)
    ]
    rg = [list(range(NCORES))]

    with tile.TileContext(nc) as tc:
        with (
            tc.tile_pool(name="const", bufs=1) as constp,
            tc.tile_pool(name="big", bufs=1) as bigp,
            tc.tile_pool(name="xt", bufs=18) as xtld,
            tc.tile_pool(name="trig", bufs=3) as trigp,
            tc.tile_pool(name="rope", bufs=3) as ropep,
            tc.tile_pool(name="pt", bufs=10) as ptp,
            tc.tile_pool(name="dinv", bufs=3) as dinvp,
            tc.tile_pool(name="hb", bufs=14) as hbp,
            tc.tile_pool(name="fout", bufs=3) as foutp,
            tc.tile_pool(name="psum", bufs=2, space="PSUM") as psp,
            tc.tile_pool(name="psum_sc", bufs=2, space="PSUM") as pssc,
            tc.tile_pool(name="psum_o", bufs=2, space="PSUM") as pso,
        ):
            # ---- constants ----
            def load_w(name, src):
                w = constp.tile([128, D], BF16, tag=name)
                nc.sync.dma_start(w[:], src[:])
                return w

            def load_w2(name, src2):
                w = constp.tile([128, D], BF16, tag=name)
                nc.scalar.dma_start(w[:], src2[:])
                return w

            wq_sb = load_w("wq", wqT)
            wk_sb = load_w2("wk", wkT)
            wv_sb = load_w("wv", wvT)
            wo_sb = load_w2("wo", woT)

            ones2_sb = constp.tile([33, 128], F32, tag="ones2")
            nc.sync.dma_start(ones2_sb[:], ones2[:])

            mask_sb = constp.tile([128, 1408], BF16, tag="mask")
            nc.sync.dma_start(mask_sb[:], masks[:])

            # ---- big persistent tiles ----
            q_sb = bigp.tile([P, S], BF16, tag="q")
            k_sb = bigp.tile([P, S], BF16, tag="k")
            # v chunks: per 128-row block B: cols [130B, 130B+130):
            #   h0 v at +0..63, h0 ones at +64, h1 v at +65..128, h1 ones at +129
            v_sb = bigp.tile([128, 130 * (S // 128)], BF16, tag="v")
            nc.vector.memset(v_sb[:], 1.0)
            ho0_sb = bigp.tile([DK, S], BF16, tag="ho0")
            ho1_sb = bigp.tile([DK, S], BF16, tag="ho1")

            # ---- projections + rope for s-tile t, as a list of ~1us chunks
            # (pumped between attention pairs to avoid lumpy PE stalls) ----
            def proj_chunks(t):
                xts = []
                for u in range(D // 128):
                    xt_t = xtld.tile([128, ST], BF16)
                    nc.sync.dma_start_transpose(
                        xt_t[:],
                        xs[ST * t : ST * (t + 1), 128 * u : 128 * (u + 1)],
                    )
                    xts.append(xt_t)

                asl = slice(ST * t, ST * (t + 1))
                ct = trigp.tile([P, ST], F32, tag="ct")
                nc.sync.dma_start(ct[:], cosg[:, asl])
                st = trigp.tile([P, ST], F32, tag="st")
                nc.sync.dma_start(st[:], sing[:, asl])

                def qk_proj(w_sb, dst):
                    pp = psp.tile([128, ST], F32, tag="proj")
                    for u in range(8):
                        nc.tensor.matmul(
                            pp[:],
                            lhsT=w_sb[:, 128 * u : 128 * (u + 1)],
                            rhs=xts[u][:],
                            start=(u == 0),
                            stop=(u == 7),
                        )
                    return pp

                def rope(pp, dst):
                    # dst = pp * cos + swap32(pp) * sin   (grouped layout)
                    pf = ropep.tile([128, ST], F32, tag="pf")
                    nc.vector.tensor_copy(pf[:], pp[:])
                    psw = ropep.tile([128, ST], F32, tag="psw")
                    for g in range(4):
                        srow = (g ^ 1) * 32
                        eng = nc.sync
                        eng.dma_start(
                            psw[32 * g : 32 * (g + 1), :],
                            pf[srow : srow + 32, :],
                        )
                    m1 = ropep.tile([128, ST], F32, tag="m1")
                    nc.vector.tensor_mul(m1[:], pp[:], ct[:])
                    m2 = ropep.tile([128, ST], F32, tag="m2")
                    nc.vector.tensor_mul(m2[:], psw[:], st[:])
                    nc.vector.tensor_add(dst[:, asl], m1[:], m2[:])

                def v_proj(sx):
                    vp = psp.tile([128, 128], F32, tag="proj")
                    for u in range(8):
                        nc.tensor.matmul(
                            vp[:],
                            lhsT=xts[u][:, 128 * sx : 128 * (sx + 1)],
                            rhs=wv_sb[:, 128 * u : 128 * (u + 1)],
                            start=(u == 0),
                            stop=(u == 7),
                        )
                    B = 4 * t + sx
                    nc.vector.tensor_copy(v_sb[:, 130 * B : 130 * B + 64], vp[:, 0:64])
                    nc.vector.tensor_copy(
                        v_sb[:, 130 * B + 65 : 130 * B + 129], vp[:, 64:128]
                    )

                state = {}
                return [
                    lambda: state.__setitem__("q", qk_proj(wq_sb, q_sb)),
                    lambda: rope(state.pop("q"), q_sb),
                    lambda: state.__setitem__("k", qk_proj(wk_sb, k_sb)),
                    lambda: rope(state.pop("k"), k_sb),
                    lambda: v_proj(0),
                    lambda: v_proj(1),
                    lambda: v_proj(2),
                    lambda: v_proj(3),
                ]

            def proj_tile(t):
                for c in proj_chunks(t):
                    c()

            # attention for one a-tile, both heads interleaved: while ACT
            # runs one head's exp, PE runs the other head's scores/PV
            def attn_tile2(A, bg=None, lowq=None):
                bg = list(bg or [])
                lowq = lowq if lowq is not None else []
                asl = slice(ST * A, ST * (A + 1))
                nB = 4 * (A + 1)
                op0 = pso.tile([65, ST], F32, tag="o")
                op1 = pso.tile([65, ST], F32, tag="o")
                ops = [op0, op1]

                def pv_pair(h, pB, pt, specs=None):
                    if specs is None:
                        specs = [(ST * i, 0, ST) for i in range(2)]
                    for i in range(2):
                        B = 2 * pB + i
                        so, ao, w = specs[i]
                        nc.tensor.matmul(
                            ops[h][:, ao : ao + w],
                            lhsT=v_sb[:, 130 * B + 65 * h : 130 * B + 65 * h + 65],
                            rhs=pt[:, so : so + w],
                            start=(B == 0),
                            stop=(B == nB - 1),
                        )

                pending = []
                for pB in range(nB // 2):
                    B0 = 2 * pB
                    diag = B0 >= 4 * A
                    dj = B0 - 4 * A
                    if not diag:
                        # full-width pair: (sp_off, q_off_in_tile, width)
                        sspec = [(ST * i, 0, ST) for i in range(2)]
                        pvspec = None
                    elif dj == 0:
                        # chunks 4A, 4A+1: windows a_local [0,512) and [128,512)
                        sspec = [(0, 0, 512), (512, 128, 384)]
                        pvspec = [(0, 0, 512), (512, 128, 384)]
                    else:
                        # chunks 4A+2, 4A+3: both over a_local [256,512)
                        sspec = [(0, 256, 256), (512, 256, 256)]
                        pvspec = [(0, 256, 256), (512, 256, 256)]
                    for h in range(2):
                        hsl = slice(DK * h, DK * (h + 1))
                        sp = pssc.tile([128, 2 * ST], F32, tag="sc")
                        for i in range(2):
                            so, ao, w = sspec[i]
                            nc.tensor.matmul(
                                sp[:, so : so + w],
                                lhsT=k_sb[hsl, 128 * (B0 + i) : 128 * (B0 + i + 1)],
                                rhs=q_sb[hsl, ST * A + ao : ST * A + ao + w],
                                start=True,
                                stop=True,
                            )
                        pt = ptp.tile([128, 2 * ST], BF16, tag="pt")
                        if not diag:
                            nc.scalar.activation(pt[:], sp[:], EXPFN)
                        elif dj == 0:
                            pte = ptp.tile([128, 2 * ST], BF16, tag="pte")
                            nc.scalar.activation(pte[:, 0:896], sp[:, 0:896], EXPFN)
                            nc.vector.tensor_mul(
                                pt[:, 0:896], pte[:, 0:896], mask_sb[:, 0:896]
                            )
                        else:
                            # two 256-wide strips at cols 0 and 512 (one bank each)
                            pte = ptp.tile([128, 2 * ST], BF16, tag="pte")
                            spv = sp[:].rearrange("p (g c) -> p g c", c=512)[:, :, 0:256]
                            ptev = pte[:].rearrange("p (g c) -> p g c", c=512)[:, :, 0:256]
                            ptv = pt[:].rearrange("p (g c) -> p g c", c=512)[:, :, 0:256]
                            mkv = mask_sb[:, 896:1408].rearrange(
                                "p (g c) -> p g c", c=256
                            )
                            nc.scalar.activation(ptev, spv, EXPFN)
                            nc.vector.tensor_mul(ptv, ptev, mkv)
                        if len(pending) >= 4:
                            pv_pair(*pending.pop(0))
                        pending.append((h, pB, pt, pvspec))
                    if bg:
                        c = bg.pop(0)
                        if c is not None:
                            c()
                    elif lowq and pB >= nB // 4:
                        c = lowq.pop(0)
                        if c is not None:
                            c()
                while pending:
                    pv_pair(*pending.pop(0))
                # proj chunks must finish before the next tile's attention
                for c in bg:
                    if c is not None:
                        c()
                dinv2 = dinvp.tile([33, ST], F32, tag="dinv")
                nc.vector.reciprocal(dinv2[0:1, :], ops[0][64:65, :])
                nc.vector.reciprocal(dinv2[32:33, :], ops[1][64:65, :])
                drep2 = psp.tile([128, ST], F32, tag="proj")
                nc.tensor.matmul(
                    drep2[:], lhsT=ones2_sb[:], rhs=dinv2[:], start=True, stop=True
                )
                for h in range(2):
                    ot = dinvp.tile([DK, ST], F32, tag="ot")
                    nc.vector.tensor_copy(ot[:], ops[h][0:64, :])
                    dst = ho0_sb if h == 0 else ho1_sb
                    nc.vector.tensor_mul(dst[:, asl], ot[:], drep2[64 * h : 64 * h + 64, :])

            # heads allgather for range k
            hb_tiles = {}

            def ho_gather(k):
                t0, ntile = RANGES[k]
                w = ntile * ST
                rsl = slice(ST * t0, ST * t0 + w)
                nc.sync.dma_start(cc_ho_in[k].ap()[0:DK, :], ho0_sb[:, rsl])
                nc.sync.dma_start(cc_ho_in[k].ap()[DK:P, :], ho1_sb[:, rsl])
                nc.gpsimd.collective_compute(
                    "AllGather",
                    mybir.AluOpType.bypass,
                    ins=[cc_ho_in[k].ap()],
                    outs=[cc_ho_out[k].ap()],
                    replica_groups=rg,
                )
                hbs = []
                for u in range(8):
                    hb = hbp.tile([128, 2 * ST], BF16, tag="hb")
                    nc.gpsimd.dma_start(
                        hb[:, :w], cc_ho_out[k].ap()[128 * u : 128 * (u + 1), :]
                    )
                    hbs.append(hb)
                hb_tiles[k] = hbs

            # out-proj matmul chunks for range k (pumped once AG_k is done)
            def outproj_chunk(k, dt_):
                t0, ntile = RANGES[k]
                t = t0 + dt_
                hbs = hb_tiles[k]
                fp = psp.tile([128, ST], F32, tag="proj")
                for u in range(8):
                    nc.tensor.matmul(
                        fp[:],
                        lhsT=wo_sb[:, 128 * u : 128 * (u + 1)],
                        rhs=hbs[u][:, ST * dt_ : ST * (dt_ + 1)],
                        start=(u == 0),
                        stop=(u == 7),
                    )
                fo = foutp.tile([128, ST], F32)
                nc.vector.tensor_copy(fo[:], fp[:])
                nc.gpsimd.dma_start(outT[:, ST * t : ST * (t + 1)], fo[:])

            def outproj_chunks(k):
                t0, ntile = RANGES[k]
                out = []
                for dt_ in range(ntile):
                    out.append(lambda d=dt_: outproj_chunk(k, d))
                out.append(lambda: hb_tiles.pop(k) and None)
                return out

            proj_tile(0)
            lowq = []  # out-proj chunks: pumped at low priority, carry across tiles
            for t in range(NT):
                bg = []
                if t + 1 < NT:
                    bg += proj_chunks(t + 1)
                if t in OUTPROJ_AT:
                    lowq += outproj_chunks(OUTPROJ_AT[t])
                attn_tile2(t, bg, lowq)
                if t in GATHER_AT:
                    ho_gather(GATHER_AT[t])
            for c in lowq + outproj_chunks(len(RANGES) - 1):
                if c is not None:
                    c()

    nc.compile()
    return nc


def _host_inputs(x, Wq, Wk, Wv, Wo):
    x2 = np.asarray(x).reshape(S, D)
    xsb = x2.astype(NPBF16)

    # grouped feature permutation per head: pos 64h+32o+f <- orig 64h+2f+o
    perm = np.empty(P, dtype=np.int64)
    for h in range(HPC):
        for o in range(2):
            for f in range(DK // 2):
                perm[DK * h + 32 * o + f] = DK * h + 2 * f + o

    pos = np.arange(S, dtype=np.float64)
    inv_freq = 1.0 / THETA ** (np.arange(0, DK, 2, dtype=np.float64) / DK)
    ang = np.outer(pos, inv_freq)  # [S, 32]
    cos32 = np.cos(ang).T.astype(np.float32)  # [32, S]
    sin32 = np.sin(ang).T.astype(np.float32)
    cosg = np.tile(cos32, (4, 1))  # [128, S] (same for E/O and both heads)
    sing = np.concatenate([-sin32, sin32, -sin32, sin32], axis=0)

    ones2 = np.zeros((33, 128), dtype=np.float32)
    ones2[0, 0:DK] = 1.0
    ones2[32, DK:128] = 1.0

    bl = np.arange(128)[:, None]
    tri = (bl <= np.arange(128)[None, :]).astype(np.float32)  # [128,128] lower-left
    on = np.ones((128, 128), dtype=np.float32)
    ze = np.zeros((128, 128), dtype=np.float32)
    # p0: j0 [tri|1|1|1] over 512, j1 [tri|1|1] over 384
    # p1: j2 [tri|1] over 256, j3 [0|tri] over 256
    mk = np.concatenate(
        [tri, on, on, on, tri, on, on, tri, on, ze, tri], axis=1
    ).astype(NPBF16)
    assert mk.shape == (128, 1408)

    scale = 1.0 / np.sqrt(DK)
    in_maps = []
    for c in range(NCORES):
        rows = slice(P * c, P * (c + 1))
        wq_c = (np.asarray(Wq)[rows][perm] * scale).astype(np.float32)
        wk_c = np.asarray(Wk)[rows][perm].astype(np.float32)
        wv_c = np.asarray(Wv)[rows]
        wo_c = np.asarray(Wo)[rows]  # output rows 128c..128c+128, all input dims
        def pack(wT):
            # wT [1024, 128] -> [128, 1024]: out[p, 128u+j] = wT[128u+p, j]
            return np.ascontiguousarray(
                wT.reshape(8, 128, 128).transpose(1, 0, 2).reshape(128, 1024)
            ).astype(NPBF16)

        in_maps.append(
            {
                "xs": xsb,
                "wqT": pack(wq_c.T),
                "wkT": pack(wk_c.T),
                "wvT": pack(wv_c.T),
                "woT": pack(wo_c.T),
                "cosg": cosg,
                "sing": sing,
                "masks": mk,
                "ones2": ones2,
            }
        )
    return in_maps


def get_program():
    if "nc" not in _CACHE:
        _CACHE["nc"] = _build_program()
    return _CACHE["nc"]


def kernel(x, Wq, Wk, Wv, Wo):
    nc = get_program()
    in_maps = _host_inputs(x, Wq, Wk, Wv, Wo)
    res = run_bass_kernel_spmd(nc, in_maps, list(range(NCORES)))
    out = np.empty((1, S, D), dtype=np.float32)
    for c in range(NCORES):
        out[0, :, P * c : P * (c + 1)] = res.results[c]["outT"].T
    return out


if __name__ == "__main__":
    import reference

    inputs = {k: np.asarray(v) for k, v in reference.setup_inputs().items()}
    got = kernel(**inputs)
    exp = np.asarray(reference.reference(**inputs))
    denom = np.abs(exp).max()
    err = np.abs(got - exp).max() / denom
    print(f"Relative error: {err:.3e}")
